# revision 21
# baseline (speedup 1.0000x reference)
"""Trainium2 Bass kernel for nn_Conditioned_Mlp (moe_routing).

Computation (reference):
    h      = relu(q @ W1[e] + b1[e])          [N, E, H]
    q_pred = h @ W2[e] + b2[e]                [N, E, D]
    gate   = softmax(concat(q, k) @ Wg + bg)  [N, E]
    out    = sum_e gate[:, e] * q_pred[:, :, e]

Sharding: pure data-parallel over N across 8 cores (2048 tokens/core);
all weights replicated.  Per core the kernel processes tokens in tiles
of 512, keeps h in transposed layout [H, tok] so layer-2 consumes it as
the stationary operand directly, and fuses gate/softmax/combine on-chip.
Matmuls run in bf16 (fp32 PSUM accumulation).

Gate path: logits are accumulated in transposed [4, tok] layout with the
tiny Wg chunks as the stationary operand (cheap LDWEIGHTS, full-width
512-token streams), exp'd with the bias folded into the activation, and
the softmax denominator is folded into the gate scalars / seed scale.

First-tile schedule: HBM cannot deliver qT+kT+W1(e0)+W2(e0) (~25 MB)
before a naive e0 layer-2 would need them (~58 us), so tile 0 runs
l1(e0) -> l1(e1) -> gate -> l2(e0) -> l2(e1), moving the kT/W2
deadlines to ~115 us; the ramp is then purely qT+W1 bound.  W2 lives as
two dh-halves so consecutive l2 blocks pipeline their weight loads.

Host-side work: dtype conversion to bf16 and weight-layout reordering so
every DMA the device issues is fully contiguous (W2 halves: 1 KB runs).
"""

import sys

sys.path.insert(0, "/opt/trn_rl_repo")

from contextlib import ExitStack

import ml_dtypes
import numpy as np

import concourse.bass as bass
import concourse.mybir as mybir
import concourse.tile as tile
from concourse import bacc
from concourse.bass import ds, ts
from concourse.bass_utils import run_bass_kernel_spmd

BF16 = mybir.dt.bfloat16
F32 = mybir.dt.float32
AF = mybir.ActivationFunctionType
ALU = mybir.AluOpType

N, D, E, H = 16384, 1024, 4, 4096
NCORES = 8
NT = N // NCORES  # tokens per core (2048)
TT = 512          # tokens per tile
NTT = NT // TT    # token tiles per core (4)
NT128 = TT // 128 # 128-token chunks per tile (4)
DC = D // 128     # contraction chunks over D (8)
HC = H // 128     # h-chunks (32)
HG = H // 512     # W1 streaming groups over H (8)

_CACHE = {}


def _build(trace_sim=False, repeat=1):
    nc = bacc.Bacc("TRN2", target_bir_lowering=False)

    # qtr[t, p, j, tok] = q[t*TT + tok, j*128 + p]  (host pre-transposed)
    q = nc.dram_tensor("qtr", [NTT, 128, DC, TT], BF16, kind="ExternalInput")
    k = nc.dram_tensor("ktr", [NTT, 128, DC, TT], BF16, kind="ExternalInput")
    # w1r[e, hg, p, d*512+s] = W1[e, d*128+p, hg*512+s]
    w1 = nc.dram_tensor("w1r", [E, HG, 128, DC * 512], BF16, kind="ExternalInput")
    # w2r[e, p, c, d] = W2[e, c*128+p, d]
    w2 = nc.dram_tensor("w2r", [E, 128, HC, D], BF16, kind="ExternalInput")
    # b1r[p, e*HC+c] = b1[e, c*128+p]
    b1 = nc.dram_tensor("b1r", [128, E * HC], F32, kind="ExternalInput")
    b2 = nc.dram_tensor("b2", [1, E * D], BF16, kind="ExternalInput")
    # wgr[p, j, g] = Wg[j*128+p, g]   (j < DC: q part; j >= DC: k part)
    wg = nc.dram_tensor("wgr", [128, 2 * DC, 4], BF16, kind="ExternalInput")
    # bgT[e, 0] = bg[e]
    bg = nc.dram_tensor("bgT", [4, 1], F32, kind="ExternalInput")
    ident = nc.dram_tensor("ident4", [4, 4], BF16, kind="ExternalInput")
    out = nc.dram_tensor("out", [NT, D], F32, kind="ExternalOutput")

    with ExitStack() as ctx:
        tc = ctx.enter_context(tile.TileContext(nc, trace_sim=trace_sim))
        const = ctx.enter_context(tc.tile_pool(name="const", bufs=1))
        qkp = ctx.enter_context(tc.tile_pool(name="qk", bufs=2))
        # kT gets a single slot: its only reader is gate_logits at tile
        # start, so the slot is free long before the e2 prefetch refills it
        kTp = ctx.enter_context(tc.tile_pool(name="kTp", bufs=1))
        w1p = ctx.enter_context(tc.tile_pool(name="w1p", bufs=4))
        w2p = ctx.enter_context(tc.tile_pool(name="w2p", bufs=1))
        htp = ctx.enter_context(tc.tile_pool(name="htp", bufs=2))
        yp = ctx.enter_context(tc.tile_pool(name="yp", bufs=1))
        gTp = ctx.enter_context(tc.tile_pool(name="gTp", bufs=2))
        gp = ctx.enter_context(tc.tile_pool(name="gp", bufs=8))
        ps1 = ctx.enter_context(tc.tile_pool(name="ps1", bufs=4, space="PSUM"))
        ps2 = ctx.enter_context(tc.tile_pool(name="ps2", bufs=4, space="PSUM"))

        # Ramp uses BOTH HWDGE rings in parallel: the SP (sync) ring
        # carries qT + the first W1 group + the hg3..7 stream; the ACT
        # (scalar) ring carries W1 groups 1-2, the consts, and kT.  Each
        # ring transfer pays ~1.5-2 us of fixed serialization, so the
        # critical path uses few, large transfers.
        w1_pre = {
            hg: w1p.tile([128, DC * 512], BF16, tag="w1", name=f"w1pre{hg}")
            for hg in range(4)
        }
        qT0 = qkp.tile([128, DC, TT], BF16, tag="qT", name="qT0")
        kT0 = kTp.tile([128, DC, TT], BF16, tag="kT", name="kT0")
        nc.sync.dma_start(out=qT0[:, 0, :], in_=q[0, :, 0, :])
        nc.sync.dma_start(out=w1_pre[0][:, :], in_=w1[0, 0, :, :])
        nc.sync.dma_start(out=qT0[:, 1:4, :], in_=q[0, :, 1:4, :])
        # the last qT chunks ride the otherwise-idle SWDGE ring (3-way
        # ring parallelism during the ramp)
        nc.gpsimd.dma_start(out=qT0[:, 4:DC, :], in_=q[0, :, 4:DC, :])

        nc.scalar.dma_start(out=w1_pre[1][:, :], in_=w1[0, 1, :, :])
        b1_sb = const.tile([128, E * HC], F32)
        nc.scalar.dma_start(out=b1_sb, in_=b1[:, :])
        nc.scalar.dma_start(out=w1_pre[2][:, :], in_=w1[0, 2, :, :])
        wg_sb = const.tile([128, 2 * DC, 4], BF16)
        nc.scalar.dma_start(out=wg_sb, in_=wg[:, :, :])
        b2_sb = const.tile([4, D], BF16)
        nc.scalar.dma_start(out=b2_sb, in_=b2[:, :].rearrange("p (e d) -> (p e) d", e=E))
        bg_sb = const.tile([4, 1], F32)
        nc.scalar.dma_start(out=bg_sb, in_=bg[:, :])
        ident_sb = const.tile([4, 4], BF16)
        nc.scalar.dma_start(out=ident_sb, in_=ident[:, :])
        nc.scalar.dma_start(out=w1_pre[3][:, :], in_=w1[0, 3, :, :])
        # kT feeds the t0 gate at ~115 us
        nc.scalar.dma_start(out=kT0[:, :, :], in_=k[0, :, :, :])

        def load_w2_halves(e, after=None):
            """W2[e] as two dh-half tiles; halves pipeline across experts.

            after: optional per-half AP whose producing DMA must land first.
            A tiny GpSimd read of it is issued before the half's DMAs; the
            SWDGE engine is FIFO, so the whole half is sequenced behind it
            (real data dependency -- sim-time waits proved unreliable).
            """
            halves = []
            for dh in range(2):
                w2h = w2p.tile([128, HC, 512], BF16, tag=f"w2_{dh}", name=f"w2h{dh}")
                if after is not None and after[dh] is not None:
                    scr = gp.tile([1, 8], BF16, tag="scr", name="scr")
                    nc.gpsimd.tensor_copy(scr, after[dh])
                for j in range(8):
                    nc.gpsimd.dma_start(
                        out=w2h[:, ds(j * 4, 4), :],
                        in_=w2[e, :, ds(j * 4, 4), ds(dh * 512, 512)],
                    )
                halves.append(w2h)
            return halves

        def l1_block(e, qT, w1_src=None, post_hg0=None, kt_interleave=None):
            """layer 1: ht[p, c, tok] = relu(q @ W1[e] + b1[e])[tok, c*128+p]"""
            ht = htp.tile([128, HC, TT], BF16, tag="ht", name="ht")
            for hg in range(HG):
                if w1_src is not None and hg in w1_src:
                    w1t = w1_src[hg]
                else:
                    w1t = w1p.tile([128, DC * 512], BF16, tag="w1", name="w1t")
                    nc.sync.dma_start(out=w1t[:, :], in_=w1[e, hg, :, :])
                if kt_interleave is not None and hg in kt_interleave:
                    kT0_, chunks = kt_interleave[hg]
                    for d in chunks:
                        nc.sync.dma_start(out=kT0_[:, d, :], in_=k[0, :, d, :])
                for hs in range(4):
                    hc = hg * 4 + hs
                    p1 = ps1.tile([128, TT], F32, tag="l1", name="p1")
                    for d in range(DC):
                        nc.tensor.matmul(
                            p1,
                            lhsT=w1t[:, ds(d * 512 + hs * 128, 128)],
                            rhs=qT[:, d, :],
                            start=(d == 0),
                            stop=(d == DC - 1),
                        )
                    nc.scalar.activation(
                        ht[:, hc, :], p1, AF.Relu,
                        bias=b1_sb[:, e * HC + hc : e * HC + hc + 1],
                    )
                if hg == 0 and post_hg0 is not None:
                    post_hg0()
            return ht

        def gate_logits(qT, kT):
            # logitsT[e, tok] accumulated with Wg chunks stationary:
            # LDWEIGHTS is 4 columns (cheap), streams are 512 wide.
            pgT = ps1.tile([4, TT], F32, tag="l1", name="pgT")
            for j in range(DC):
                nc.tensor.matmul(
                    pgT, lhsT=wg_sb[:, j, :], rhs=qT[:, j, :],
                    start=(j == 0), stop=False,
                )
            for j in range(DC):
                nc.tensor.matmul(
                    pgT, lhsT=wg_sb[:, DC + j, :], rhs=kT[:, j, :],
                    start=False, stop=(j == DC - 1),
                )
            gT = gTp.tile([4, TT], BF16, tag="gexpT", name="gT")
            # logits ~N(0,1); exp cannot overflow, skip max-subtraction
            nc.scalar.activation(gT, pgT, AF.Exp, bias=bg_sb[:, :])
            return gT

        def gate_finish(gexpT, y):
            # Per-t4: transpose exp(logits) to token-partition layout,
            # row-sum on the evacuation copy, normalize.  Also seed y with
            # the gate-weighted b2 (denominator folded into the copy) so
            # per-expert layer-2 paths skip their bias entirely.
            gates, recips = [], []
            for t4 in range(NT128):
                pt = ps1.tile([128, 4], BF16, tag="l1", name="pt")
                nc.tensor.transpose(pt, gexpT[:, ts(t4, 128)], ident_sb)
                gexp = gp.tile([128, 4], F32, tag="gexp", name="gexp")
                gsum = gp.tile([128, 1], F32, tag="gsum", name="gsum")
                nc.scalar.activation(gexp, pt, AF.Copy, accum_out=gsum)
                grec = gp.tile([128, 1], F32, tag="grec", name="grec")
                nc.vector.reciprocal(grec, gsum)
                gate = gp.tile([128, 4], F32, tag="gate", name="gate")
                nc.vector.tensor_scalar_mul(gate, gexp, grec)
                gates.append(gate)
                recips.append(grec)
            for dh in range(2):
                for t4 in range(NT128):
                    pb = ps2.tile([128, 512], F32, tag="l2", name="pb")
                    nc.tensor.matmul(
                        pb, lhsT=gexpT[:, ts(t4, 128)],
                        rhs=b2_sb[:, ds(dh * 512, 512)],
                        start=True, stop=True,
                    )
                    nc.scalar.activation(
                        y[:, t4, ds(dh * 512, 512)], pb, AF.Copy,
                        scale=recips[t4][:, :],
                    )
            return gates

        def l2_block(e, ht, w2h, gates, y, tok0, last):
            # layer 2 + gated accumulation into y.  t4-inner keeps all 32
            # accumulation matmuls on ONE psum bank back-to-back --
            # per-instruction bank cycling triggers the documented HAM
            # micro-idle oscillation (~45% PE throughput loss).
            for dh in range(2):
                for t4 in range(NT128):
                    p2 = ps2.tile([128, 512], F32, tag="l2", name="p2")
                    for h in range(HC):
                        nc.tensor.matmul(
                            p2,
                            lhsT=ht[:, h, ts(t4, 128)],
                            rhs=w2h[dh][:, h, :],
                            start=(h == 0),
                            stop=(h == HC - 1),
                        )
                    g_col = gates[t4][:, e : e + 1]
                    ysl = y[:, t4, ds(dh * 512, 512)]
                    nc.vector.scalar_tensor_tensor(
                        out=ysl, in0=p2, scalar=g_col, in1=ysl,
                        op0=ALU.mult, op1=ALU.add,
                    )
                    if last:
                        # stream out per (dh, t4) as soon as it lands
                        nc.scalar.dma_start(
                            out=out[
                                tok0 + t4 * 128 : tok0 + (t4 + 1) * 128,
                                ds(dh * 512, 512),
                            ],
                            in_=ysl,
                        )

        for _rep in range(repeat):
          qk_next = (qT0, kT0)

          # ---- tile 0: l1(e0) l1(e1) gate l2(e0) l2(e1), then e2/e3 --
          # moves the kT / W2 deadlines from ~58 us to ~115 us so the ramp
          # is purely qT+W1 bound.  W2(e0) halves are sequenced behind the
          # last qT / kT chunk landing so they can't starve the ramp.
          tok0 = 0
          qT, kT = qk_next
          y = yp.tile([128, NT128, D], F32, tag="y", name="y")
          w2h0 = load_w2_halves(
              0, after=(qT0[0:1, DC - 1, 0:8], kT0[0:1, DC - 1, 0:8])
          )
          ht0 = l1_block(0, qT, w1_src=w1_pre)
          w2h1 = load_w2_halves(1)
          ht1 = l1_block(1, qT)
          gexpT = gate_logits(qT, kT)
          gates = gate_finish(gexpT, y)
          l2_block(0, ht0, w2h0, gates, y, tok0, last=False)
          l2_block(1, ht1, w2h1, gates, y, tok0, last=False)
          for e in (2, 3):
              w2h = load_w2_halves(e)
              if e == 2:
                  qTn = qkp.tile([128, DC, TT], BF16, tag="qT", name="qTn")
                  kTn = kTp.tile([128, DC, TT], BF16, tag="kT", name="kTn")
                  nc.scalar.dma_start(out=qTn[:, :, :], in_=q[1, :, :, :])
                  nc.scalar.dma_start(out=kTn[:, :, :], in_=k[1, :, :, :])
                  qk_next = (qTn, kTn)
              ht = l1_block(e, qT)
              l2_block(e, ht, w2h, gates, y, tok0, last=(e == E - 1))

          # ---- steady-state tiles
          for t in range(1, NTT):
              tok0 = t * TT
              qT, kT = qk_next
              y = yp.tile([128, NT128, D], F32, tag="y", name="y")
              # qT/kT prefetched long ago: compute the gate logits up
              # front so the finish block never stalls the PE later.
              gexpT = gate_logits(qT, kT)
              gates = None

              for e in range(E):
                  w2h = load_w2_halves(e)
                  if e == 2 and t + 1 < NTT:
                      # prefetch next token tile's transposed q/k
                      # mid-expert, away from tile-boundary windows
                      qTn = qkp.tile([128, DC, TT], BF16, tag="qT", name="qTn")
                      kTn = kTp.tile([128, DC, TT], BF16, tag="kT", name="kTn")
                      nc.scalar.dma_start(out=qTn[:, :, :], in_=q[t + 1, :, :, :])
                      nc.scalar.dma_start(out=kTn[:, :, :], in_=k[t + 1, :, :, :])
                      qk_next = (qTn, kTn)
                  post = None
                  if e == 0:
                      # finish the gate while layer-1 streams (exp(logitsT)
                      # is ready by hg0's end: zero PE head-of-line wait)
                      def post():
                          nonlocal gates
                          gates = gate_finish(gexpT, y)
                  ht = l1_block(e, qT, post_hg0=post)
                  l2_block(e, ht, w2h, gates, y, tok0, last=(e == E - 1))

    nc.compile()
    return nc


def _get_nc():
    if "nc" not in _CACHE:
        _CACHE["nc"] = _build()
    return _CACHE["nc"]


def _prep_inputs(q, k, W1, b1, W2, b2, Wg, bg):
    bf16 = ml_dtypes.bfloat16
    q = np.asarray(q, dtype=np.float32)
    k = np.asarray(k, dtype=np.float32)
    W1 = np.asarray(W1, dtype=np.float32)
    b1 = np.asarray(b1, dtype=np.float32)
    W2 = np.asarray(W2, dtype=np.float32)
    b2 = np.asarray(b2, dtype=np.float32)
    Wg = np.asarray(Wg, dtype=np.float32)
    bg = np.asarray(bg, dtype=np.float32)

    # per-core pre-transposed q/k: [NTT, 128, DC, TT]
    def tr(x):
        xc = x.astype(bf16).reshape(NCORES, NTT, TT, DC, 128)
        return np.ascontiguousarray(xc.transpose(0, 1, 4, 3, 2))

    qtr = tr(q)
    ktr = tr(k)
    w1r = np.ascontiguousarray(
        W1.astype(bf16).reshape(E, DC, 128, HG, 512).transpose(0, 3, 2, 1, 4)
    ).reshape(E, HG, 128, DC * 512)
    w2r = np.ascontiguousarray(
        W2.astype(bf16).reshape(E, HC, 128, D).transpose(0, 2, 1, 3)
    )
    b1r = np.ascontiguousarray(
        b1.reshape(E, HC, 128).transpose(2, 0, 1).reshape(128, E * HC)
    )
    wgr = np.ascontiguousarray(
        Wg.astype(bf16).reshape(2 * DC, 128, 4).transpose(1, 0, 2)
    )
    bgT = np.ascontiguousarray(bg.astype(np.float32).reshape(4, 1))

    in_maps = []
    for c in range(NCORES):
        in_maps.append(
            {
                "qtr": qtr[c],
                "ktr": ktr[c],
                "w1r": w1r,
                "w2r": w2r,
                "b1r": b1r,
                "b2": np.ascontiguousarray(b2.astype(bf16).reshape(1, E * D)),
                "wgr": wgr,
                "bgT": bgT,
                "ident4": np.eye(4, dtype=bf16),
            }
        )
    return in_maps


def run(inputs, trace=False):
    """Run the kernel; returns (output, BassKernelResults)."""
    in_maps = _prep_inputs(**inputs)
    res = run_bass_kernel_spmd(
        _get_nc(), in_maps, core_ids=list(range(NCORES)), trace=trace
    )
    out = np.concatenate([r["out"] for r in res.results], axis=0)
    return out, res


def kernel(**inputs):
    out, _ = run(inputs, trace=False)
    return out
